# revision 8
# baseline (speedup 1.0000x reference)
"""Pairwise IoU (8192x8192) on 8 Trainium2 NeuronCores via Bass/Tile.

Strategy
--------
boxes1 rows are split across 8 cores (1024 sorted rows each).  Both box
sets are sorted by x1 on the host; boxes with x-ranges further apart
than the max box width cannot overlap, so each 128-row i-tile only has
to be scored against a contiguous window of ~2.4k x1-sorted boxes2
columns (~29% of the 8192).  The rest of the output is exactly zero and
is assembled on the host.

The device program is SPMD (one program, 8 cores), so the per-i-tile
window offsets are fixed compile-time constants OFFS[t]; the host packs
each core's column window so that tile t's true window lies inside
[base_c + OFFS[t], base_c + OFFS[t] + W).  OFFS and W are derived from
the actual data at first call.

Per-core device kernel, per [128, W] output tile:
  rx    = relu(min(x2_i, X2_j) - max(x1_i, X1_j))   custom DVE op (fp16 out)
  ry    = same for y                                custom DVE op (fp16 out)
  inter = rx*ry                                     DVE tensor_tensor (fp16, 2x)
  u     = a1_i + a2e_j - inter                      TensorE (fp16 rank-2 +
                                                    (-I)@inter) -> PSUM fp32
  rinv  = Exp(-Ln(u))                               ScalarE LUTs (fp16 out)
  out   = inter * rinv                              DVE tensor_tensor (bf16 out)
"""

import numpy as np

N = 8192
M = 8192
NCORES = 8
ROWS = N // NCORES  # rows of boxes1 per core
P = 128  # partitions
NT = ROWS // P  # 8 i-tiles per core
PS = 512  # psum bank width (fp32)
EPS = 1e-7

# 1-Newton reciprocal constants (fallback DVE div path)
RC0 = -0.23549792
RC1 = 2.0017324

USE_SCALAR_DIV = True  # TensorE union + ScalarE ln/exp + DVE mul
GPSIMD_INTER = True  # compute inter on GpSimd instead of DVE

_COMPILED = {}


def _register_op(name, spec, subdim=False):
    import concourse.dve_ops as dve_ops
    from concourse.dve_spec import lower
    from concourse.dve_uop import DveOpSpec

    for op in dve_ops.OPS:
        if op.name == name:
            return op
    shas = {}
    for ver in ("v3", "v4"):
        try:
            shas[ver] = DveOpSpec(
                name=name, opcode=0, uops=lower(spec, ver=ver)
            ).sha(ver)
        except Exception:
            pass
    op = dve_ops.DveOp(name, spec, subdim=subdim, uops_sha=shas)
    dve_ops.OPS.append(op)
    dve_ops.CUSTOM_DVE_SPECS[op.name] = op.spec
    dve_ops._SUB_OPCODE_FOR_NAME[op.name] = (
        dve_ops._CUSTOM_DVE_ROW_BASE + len(dve_ops.OPS) - 1
    )
    return op


def _np_recip1(u):
    nu = (~np.asarray(u, np.float32).view(np.int32)).view(np.float32)
    y0 = (nu * np.float32(RC0)).astype(np.float32)
    return (y0 * (np.float32(RC1) - u * y0)).astype(np.float32)


def _ensure_ops():
    """Register the IOU_EDGE and IOU_DIV custom DVE ops (idempotent)."""
    from concourse.dve_spec import (
        C0,
        C1,
        C2,
        AluOp,
        Bin,
        Spec,
        Src0,
        Src1,
        maxx,
        minn,
        relu,
    )

    edge = _register_op(
        "IOU_EDGE",
        Spec(
            body=relu(minn(Src1, C1) - maxx(Src0, C0)),
            reference=lambda in0, in1, s0, s1, imm2: np.maximum(
                np.minimum(in1, s1) - np.maximum(in0, s0), 0.0
            ).astype(np.float32),
        ),
    )

    # u = (a1 + a2e) - inter; out = inter * recip1NR(u).  8 ALU stages.
    _t1 = C0 + Src1
    _u = _t1 - Src0
    _nu = Bin(AluOp.BITWISE_NOT, _u, _u)
    _y0 = _nu * C1
    _y1 = _y0 * (C2 - _u * _y0)
    div = _register_op(
        "IOU_DIV",
        Spec(
            body=Src0 * _y1,
            reference=lambda in0, in1, s0, s1, imm2: (
                in0 * _np_recip1((s0 + in1) - in0)
            ).astype(np.float32),
        ),
    )
    return edge, div


def _build_program(W, OFFS, WCOL):
    from contextlib import ExitStack

    import concourse.bacc as bacc
    import concourse.mybir as mybir
    import concourse.tile as tile

    iou_edge, iou_div = _ensure_ops()

    f32 = mybir.dt.float32
    f16 = mybir.dt.float16
    bf16 = mybir.dt.bfloat16
    act = mybir.ActivationFunctionType
    nc = bacc.Bacc(
        "TRN2",
        target_bir_lowering=False,
        debug=False,
        enable_asserts=False,
        num_devices=NCORES,
    )

    if USE_SCALAR_DIV:
        # The default act-table placement resolves Ln and Exp to different
        # table sets, reloading tables on every switch (~2.7us each).  Route
        # both to the one set that contains them, preserving set indices.
        import types

        import bass_rust as _bass_rust
        from concourse.hw_specs import get_activation_tables

        def _insert_act_table_loads(self):
            has_activation = any(
                isinstance(i, mybir.InstActivation)
                for b in self.main_func.blocks
                for i in b.instructions
            )
            if not has_activation:
                return
            both = {act.Ln, act.Exp}
            tables = [
                (name, fns if both <= fns else fns - both)
                for name, fns in get_activation_tables(self.m.arch).items()
            ]
            _bass_rust.insert_act_table_loads(self, tables)

        nc.insert_act_table_loads = types.MethodType(_insert_act_table_loads, nc)

    # DRAM I/O. boxes2 coord rows are host-replicated across partitions.
    x1b = nc.dram_tensor("x1b", [P, WCOL], f32, kind="ExternalInput").ap()
    x2b = nc.dram_tensor("x2b", [P, WCOL], f32, kind="ExternalInput").ap()
    y1b = nc.dram_tensor("y1b", [P, WCOL], f32, kind="ExternalInput").ap()
    y2b = nc.dram_tensor("y2b", [P, WCOL], f32, kind="ExternalInput").ap()
    # Per-partition scalars: for i-tile t, columns t*5+k hold
    # (x1, x2, y1, y2, area1) of sorted boxes1 row t*128+p.
    sc = nc.dram_tensor("sc", [P, NT * 5], f32, kind="ExternalInput").ap()
    if USE_SCALAR_DIV:
        # moving operand for the union matmul: row0 = ones, row1 = a2+eps
        a2e2 = nc.dram_tensor("a2e2", [2, WCOL], f16, kind="ExternalInput").ap()
        # stationary: row0 = area1 (per sorted row), row1 = ones
        sta = nc.dram_tensor("sta", [2, ROWS], f16, kind="ExternalInput").ap()
        negi = nc.dram_tensor("negi", [P, P], f16, kind="ExternalInput").ap()
    else:
        a2eb = nc.dram_tensor("a2eb", [P, WCOL], f16, kind="ExternalInput").ap()
    out = nc.dram_tensor("out", [ROWS, W], bf16, kind="ExternalOutput").ap()

    NCH = -(-W // PS)  # psum chunks per tile

    with tile.TileContext(nc) as tc, ExitStack() as ctx:
        bc = ctx.enter_context(tc.tile_pool(name="bc", bufs=1))
        scp = ctx.enter_context(tc.tile_pool(name="scp", bufs=1))
        work = ctx.enter_context(tc.tile_pool(name="work", bufs=2))
        outp = ctx.enter_context(tc.tile_pool(name="outp", bufs=3))
        if USE_SCALAR_DIV:
            psum = ctx.enter_context(
                tc.tile_pool(name="psum", bufs=1, space="PSUM")
            )

        sct = scp.tile([P, NT * 5], f32)
        x1t = bc.tile([P, WCOL], f32)
        x2t = bc.tile([P, WCOL], f32)
        y1t = bc.tile([P, WCOL], f32)
        y2t = bc.tile([P, WCOL], f32)
        if USE_SCALAR_DIV:
            a2e2t = scp.tile([2, WCOL], f16)
            stat = scp.tile([2, ROWS], f16)
            negit = scp.tile([P, P], f16)
        else:
            a2et = bc.tile([P, WCOL], f16)

        # Load order: tile-0 windows of x (then y) coords first so compute
        # starts early; tails follow.  Spread the two head loads across the
        # two HWDGE queues (sync + scalar).
        nc.sync.dma_start(sct[:], sc[:])
        nc.scalar.dma_start(x2t[:, :W], x2b[:, :W])
        nc.sync.dma_start(x1t[:, :W], x1b[:, :W])
        nc.scalar.dma_start(y2t[:, :W], y2b[:, :W])
        nc.sync.dma_start(y1t[:, :W], y1b[:, :W])
        if USE_SCALAR_DIV:
            nc.scalar.dma_start(stat[:], sta[:])
            nc.scalar.dma_start(negit[:], negi[:])
            nc.scalar.dma_start(a2e2t[:], a2e2[:])
        else:
            nc.scalar.dma_start(a2et[:], a2eb[:])
        if WCOL > W:
            nc.sync.dma_start(x1t[:, W:], x1b[:, W:])
            nc.scalar.dma_start(x2t[:, W:], x2b[:, W:])
            nc.sync.dma_start(y1t[:, W:], y1b[:, W:])
            nc.scalar.dma_start(y2t[:, W:], y2b[:, W:])

        for t in range(NT):
            o = OFFS[t]
            c = t * 5
            rx = work.tile([P, W], f16, tag="rx")
            ry = work.tile([P, W], f16, tag="ry")
            inter = work.tile([P, W], f16, tag="inter")
            ot = outp.tile([P, W], bf16, tag="ot")

            nc.vector._custom_dve(
                iou_edge,
                out=rx[:],
                in0=x1t[:, o : o + W],
                in1=x2t[:, o : o + W],
                s0=sct[:, c : c + 1],
                s1=sct[:, c + 1 : c + 2],
            )
            nc.vector._custom_dve(
                iou_edge,
                out=ry[:],
                in0=y1t[:, o : o + W],
                in1=y2t[:, o : o + W],
                s0=sct[:, c + 2 : c + 3],
                s1=sct[:, c + 3 : c + 4],
            )
            if GPSIMD_INTER:
                nc.gpsimd.tensor_mul(inter[:], rx[:], ry[:])
            else:
                nc.vector.tensor_mul(inter[:], rx[:], ry[:])

            if USE_SCALAR_DIV:
                ua = work.tile([P, W], f32, tag="ua")
                rinv = work.tile([P, W], f16, tag="rinv")
                pts = []
                for k in range(NCH):
                    c0 = k * PS
                    c1 = min(W, c0 + PS)
                    pt = psum.tile([P, PS], f32, tag="pt", bufs=8)
                    pt = pt[:, : c1 - c0]
                    pts.append((pt, c0, c1))
                    nc.tensor.matmul(
                        pt[:],
                        stat[:, t * P : (t + 1) * P],
                        a2e2t[:, o + c0 : o + c1],
                        start=True,
                        stop=False,
                    )
                for pt, c0, c1 in pts:
                    nc.tensor.matmul(
                        pt[:],
                        negit[:],
                        inter[:, c0:c1],
                        start=False,
                        stop=True,
                    )
                for pt, c0, c1 in pts:
                    nc.scalar.activation(ua[:, c0:c1], pt[:], act.Ln)
                nc.scalar.activation(rinv[:], ua[:], act.Exp, scale=-1.0)
                nc.vector.tensor_mul(ot[:], inter[:], rinv[:])
            else:
                nc.vector._custom_dve(
                    iou_div,
                    out=ot[:],
                    in0=inter[:],
                    in1=a2et[:, o : o + W],
                    s0=sct[:, c + 4 : c + 5],
                    s1=RC0,
                    imm2=RC1,
                )
            nc.sync.dma_start(out[t * P : (t + 1) * P, :], ot[:])

    nc.compile()
    return nc


def _get_program(W, OFFS, WCOL):
    key = (W, tuple(OFFS), WCOL)
    if key not in _COMPILED:
        _COMPILED[key] = _build_program(W, list(OFFS), WCOL)
    return _COMPILED[key]


def _plan(boxes1, boxes2):
    """Sort boxes, derive per-tile column windows and the OFFS/W packing."""
    b1 = np.ascontiguousarray(boxes1, dtype=np.float32)
    b2 = np.ascontiguousarray(boxes2, dtype=np.float32)
    p1 = np.argsort(b1[:, 0], kind="stable")
    p2 = np.argsort(b2[:, 0], kind="stable")
    s1 = b1[p1]
    s2 = b2[p2]
    X1 = s2[:, 0]
    wmax2 = float((s2[:, 2] - s2[:, 0]).max())

    jL = np.empty((NCORES, NT), np.int64)
    jR = np.empty((NCORES, NT), np.int64)
    for c in range(NCORES):
        for t in range(NT):
            rows = s1[c * ROWS + t * P : c * ROWS + (t + 1) * P]
            lo = float(rows[:, 0].min())
            hi = float(rows[:, 2].max())
            jL[c, t] = np.searchsorted(X1, np.float32(lo - wmax2) - 1e-3)
            jR[c, t] = np.searchsorted(X1, np.float32(hi) + 1e-3)

    def wneed(offs):
        l = jL - offs[None, :]
        r = jR - offs[None, :]
        return int((r.max(axis=1) - l.min(axis=1)).max())

    ts = np.arange(NT)
    best = None
    for S in range(0, 513, 16):
        Wn = wneed(S * ts)
        if best is None or Wn < best[0]:
            best = (Wn, S * ts)
    # refine: per-tile offsets at the cross-core median of jL (even-rounded)
    med = np.median(jL - jL[:, :1], axis=0)
    cand = 2 * np.round((med - med.min()) / 2).astype(np.int64)
    Wn = wneed(cand)
    if Wn < best[0]:
        best = (Wn, cand)
    Wneed, offs = best
    W = min(-(-max(Wneed, 64) // 32) * 32, M + 512)
    offs = offs - offs.min()
    WCOL = int(offs.max()) + W
    bases = (jL - offs[None, :]).min(axis=1)  # per-core packed origin
    return dict(
        b1=b1, b2=b2, p1=p1, p2=p2, s1=s1, s2=s2,
        W=W, OFFS=[int(o) for o in offs], WCOL=WCOL, bases=bases,
    )


def _make_in_maps(plan):
    s1, s2 = plan["s1"], plan["s2"]
    W, OFFS, WCOL, bases = plan["W"], plan["OFFS"], plan["WCOL"], plan["bases"]

    X1, Y1, X2, Y2 = s2[:, 0], s2[:, 1], s2[:, 2], s2[:, 3]
    a2e = ((X2 - X1) * (Y2 - Y1) + np.float32(EPS)).astype(np.float32)

    in_maps = []
    for c in range(NCORES):
        idx = bases[c] + np.arange(WCOL)
        valid = (idx >= 0) & (idx < M)
        idxc = np.clip(idx, 0, M - 1)
        pad = np.float32(-1e6)

        def rep(vec, fill, dt=np.float32):
            row = np.where(valid, vec[idxc], fill).astype(dt)
            return np.ascontiguousarray(np.broadcast_to(row, (P, WCOL)))

        m = {
            "x1b": rep(X1, pad),
            "x2b": rep(X2, pad),
            "y1b": rep(Y1, pad),
            "y2b": rep(Y2, pad),
        }
        rows = s1[c * ROWS : (c + 1) * ROWS].reshape(NT, P, 4)
        a1 = (rows[:, :, 2] - rows[:, :, 0]) * (rows[:, :, 3] - rows[:, :, 1])
        scv = np.empty((P, NT * 5), dtype=np.float32)
        for t in range(NT):
            scv[:, t * 5 + 0] = rows[t, :, 0]
            scv[:, t * 5 + 1] = rows[t, :, 2]
            scv[:, t * 5 + 2] = rows[t, :, 1]
            scv[:, t * 5 + 3] = rows[t, :, 3]
            scv[:, t * 5 + 4] = a1[t]
        m["sc"] = scv
        if USE_SCALAR_DIV:
            a2row = np.where(valid, a2e[idxc], np.float32(1.0))
            m["a2e2"] = np.ascontiguousarray(
                np.stack([np.ones(WCOL, np.float32), a2row]).astype(np.float16)
            )
            m["sta"] = np.ascontiguousarray(
                np.stack([a1.reshape(ROWS), np.ones(ROWS, np.float32)]).astype(
                    np.float16
                )
            )
            m["negi"] = (-np.eye(P)).astype(np.float16)
        else:
            m["a2eb"] = rep(a2e, np.float32(1.0), np.float16)
        in_maps.append(m)
    return in_maps


def _assemble(plan, results):
    """Paste per-core [ROWS, W] bf16 blocks into the full fp32 matrix."""
    W, OFFS, bases = plan["W"], plan["OFFS"], plan["bases"]
    p1, p2 = plan["p1"], plan["p2"]

    out_sorted = np.zeros((N, M), dtype=np.float32)
    for c in range(NCORES):
        blk = np.asarray(results[c]["out"])
        for t in range(NT):
            c0 = bases[c] + OFFS[t]
            c1 = c0 + W
            s0 = max(0, -c0)
            cc0 = max(0, c0)
            cc1 = min(M, c1)
            if cc1 <= cc0:
                continue
            out_sorted[
                c * ROWS + t * P : c * ROWS + (t + 1) * P, cc0:cc1
            ] = blk[t * P : (t + 1) * P, s0 : s0 + (cc1 - cc0)].astype(
                np.float32
            )

    inv1 = np.empty(N, np.int64)
    inv1[p1] = np.arange(N)
    inv2 = np.empty(M, np.int64)
    inv2[p2] = np.arange(M)
    tmp = out_sorted[inv1]
    return np.take(tmp, inv2, axis=1)


def _run(inputs, trace=False, tmpdir=None):
    from concourse.bass_utils import run_bass_kernel_spmd

    plan = _plan(inputs["boxes1"], inputs["boxes2"])
    nc = _get_program(plan["W"], plan["OFFS"], plan["WCOL"])
    in_maps = _make_in_maps(plan)
    kwargs = {}
    if trace:
        kwargs = dict(trace=True, tmpdir=tmpdir)
    res = run_bass_kernel_spmd(
        nc, in_maps, core_ids=list(range(NCORES)), **kwargs
    )
    return plan, res


def kernel(boxes1: np.ndarray, boxes2: np.ndarray) -> np.ndarray:
    plan, res = _run({"boxes1": boxes1, "boxes2": boxes2})
    return _assemble(plan, res.results)


# revision 9
# speedup vs baseline: 1.2878x; 1.2878x over previous
"""Pairwise IoU (8192x8192) on 8 Trainium2 NeuronCores via Bass/Tile.

Strategy
--------
boxes1 rows are split across 8 cores (1024 sorted rows each).  Both box
sets are sorted by x1 on the host; boxes with x-ranges further apart
than the max box width cannot overlap, so each 128-row i-tile only has
to be scored against a contiguous window of ~2.3k x1-sorted boxes2
columns (~28% of the 8192).  The rest of the output is exactly zero and
is assembled on the host.

The device program is SPMD (one program, 8 cores), so the per-i-tile
window offsets/widths are fixed compile-time constants OFFS[t]/WT[t];
the host packs each core's column window so that tile t's true window
lies inside [base_c + OFFS[t], base_c + OFFS[t] + WT[t]).  All derived
from the actual data at first call.

Per-core device kernel, per [128, WT] output tile:
  rx    = relu(min(x2_i, X2_j) - max(x1_i, X1_j))   custom DVE op (fp16 out)
  ry    = same for y                                custom DVE op (fp16 out)
  inter = rx*ry                                     DVE tensor_tensor (fp16, 2x)
  p     = a2e_j - inter                             TensorE (ones rank-1 +
                                                    (-I)@inter, fp16) -> PSUM
  rinv  = Exp(-Ln(p + a1_i))                        ScalarE LUTs (a1 via the
                                                    per-partition Ln bias)
  out   = inter * rinv                              DVE tensor_tensor (bf16 out)
"""

import numpy as np

N = 8192
M = 8192
NCORES = 8
ROWS = N // NCORES  # rows of boxes1 per core
P = 128  # partitions
NT = ROWS // P  # 8 i-tiles per core
PS = 512  # psum bank width (fp32)
EPS = 1e-7

_COMPILED = {}


def _register_op(name, spec, subdim=False):
    import concourse.dve_ops as dve_ops
    from concourse.dve_spec import lower
    from concourse.dve_uop import DveOpSpec

    for op in dve_ops.OPS:
        if op.name == name:
            return op
    shas = {}
    for ver in ("v3", "v4"):
        try:
            shas[ver] = DveOpSpec(
                name=name, opcode=0, uops=lower(spec, ver=ver)
            ).sha(ver)
        except Exception:
            pass
    op = dve_ops.DveOp(name, spec, subdim=subdim, uops_sha=shas)
    dve_ops.OPS.append(op)
    dve_ops.CUSTOM_DVE_SPECS[op.name] = op.spec
    dve_ops._SUB_OPCODE_FOR_NAME[op.name] = (
        dve_ops._CUSTOM_DVE_ROW_BASE + len(dve_ops.OPS) - 1
    )
    return op


def _ensure_ops():
    """Register the IOU_EDGE custom DVE op (idempotent)."""
    from concourse.dve_spec import C0, C1, Spec, Src0, Src1, maxx, minn, relu

    return _register_op(
        "IOU_EDGE",
        Spec(
            body=relu(minn(Src1, C1) - maxx(Src0, C0)),
            reference=lambda in0, in1, s0, s1, imm2: np.maximum(
                np.minimum(in1, s1) - np.maximum(in0, s0), 0.0
            ).astype(np.float32),
        ),
    )


def _build_program(WT, OFFS, WCOL):
    from contextlib import ExitStack

    import concourse.bacc as bacc
    import concourse.mybir as mybir
    import concourse.tile as tile

    iou_edge = _ensure_ops()

    f32 = mybir.dt.float32
    f16 = mybir.dt.float16
    bf16 = mybir.dt.bfloat16
    act = mybir.ActivationFunctionType
    nc = bacc.Bacc(
        "TRN2",
        target_bir_lowering=False,
        debug=False,
        enable_asserts=False,
        num_devices=NCORES,
    )

    # The default act-table placement resolves Ln and Exp to different
    # table sets, reloading tables on every switch (~2.7us each).  Route
    # both to the one set that contains them, preserving set indices.
    import types

    import bass_rust as _bass_rust
    from concourse.hw_specs import get_activation_tables

    def _insert_act_table_loads(self):
        has_activation = any(
            isinstance(i, mybir.InstActivation)
            for b in self.main_func.blocks
            for i in b.instructions
        )
        if not has_activation:
            return
        both = {act.Ln, act.Exp}
        tables = [
            (name, fns if both <= fns else fns - both)
            for name, fns in get_activation_tables(self.m.arch).items()
        ]
        _bass_rust.insert_act_table_loads(self, tables)

    nc.insert_act_table_loads = types.MethodType(_insert_act_table_loads, nc)

    WMAX = max(WT)
    W0 = WT[0]
    H0 = (W0 // 2 + 31) & ~31  # first-tile split point

    # DRAM I/O. boxes2 coord rows are host-replicated across partitions.
    x1b = nc.dram_tensor("x1b", [P, WCOL], f32, kind="ExternalInput").ap()
    x2b = nc.dram_tensor("x2b", [P, WCOL], f32, kind="ExternalInput").ap()
    y1b = nc.dram_tensor("y1b", [P, WCOL], f32, kind="ExternalInput").ap()
    y2b = nc.dram_tensor("y2b", [P, WCOL], f32, kind="ExternalInput").ap()
    # Per-partition scalars: for i-tile t, columns t*5+k hold
    # (x1, x2, y1, y2, area1) of sorted boxes1 row t*128+p.
    sc = nc.dram_tensor("sc", [P, NT * 5], f32, kind="ExternalInput").ap()
    a2e1 = nc.dram_tensor("a2e1", [1, WCOL], f16, kind="ExternalInput").ap()
    ones1 = nc.dram_tensor("ones1", [1, P], f16, kind="ExternalInput").ap()
    negi = nc.dram_tensor("negi", [P, P], f16, kind="ExternalInput").ap()
    out = nc.dram_tensor("out", [ROWS, WMAX], bf16, kind="ExternalOutput").ap()

    with tile.TileContext(nc) as tc, ExitStack() as ctx:
        bc = ctx.enter_context(tc.tile_pool(name="bc", bufs=1))
        scp = ctx.enter_context(tc.tile_pool(name="scp", bufs=1))
        work = ctx.enter_context(tc.tile_pool(name="work", bufs=2))
        outp = ctx.enter_context(tc.tile_pool(name="outp", bufs=3))
        psum = ctx.enter_context(tc.tile_pool(name="psum", bufs=1, space="PSUM"))

        sct = scp.tile([P, NT * 5], f32)
        x1t = bc.tile([P, WCOL], f32)
        x2t = bc.tile([P, WCOL], f32)
        y1t = bc.tile([P, WCOL], f32)
        y2t = bc.tile([P, WCOL], f32)
        a2e1t = scp.tile([1, WCOL], f16)
        ones1t = scp.tile([1, P], f16)
        negit = scp.tile([P, P], f16)

        # Load order: tile-0's window (in two chunks so its first edge op
        # can start after ~H0 columns land), then tails.  Two HWDGE queues.
        nc.sync.dma_start(sct[:], sc[:])
        nc.scalar.dma_start(x2t[:, :H0], x2b[:, :H0])
        nc.sync.dma_start(x1t[:, :H0], x1b[:, :H0])
        nc.scalar.dma_start(x2t[:, H0:W0], x2b[:, H0:W0])
        nc.sync.dma_start(x1t[:, H0:W0], x1b[:, H0:W0])
        nc.scalar.dma_start(y2t[:, :W0], y2b[:, :W0])
        nc.sync.dma_start(y1t[:, :W0], y1b[:, :W0])
        nc.scalar.dma_start(negit[:], negi[:])
        nc.scalar.dma_start(ones1t[:], ones1[:])
        nc.scalar.dma_start(a2e1t[:], a2e1[:])
        nc.sync.dma_start(x1t[:, W0:], x1b[:, W0:])
        nc.scalar.dma_start(x2t[:, W0:], x2b[:, W0:])
        nc.sync.dma_start(y1t[:, W0:], y1b[:, W0:])
        nc.scalar.dma_start(y2t[:, W0:], y2b[:, W0:])

        for t in range(NT):
            o = OFFS[t]
            c = t * 5
            W = WT[t]
            rx = work.tile([P, WMAX], f16, tag="rx")
            ry = work.tile([P, WMAX], f16, tag="ry")
            inter = work.tile([P, WMAX], f16, tag="inter")
            ua = work.tile([P, WMAX], f32, tag="ua")
            rinv = work.tile([P, WMAX], f16, tag="rinv")
            ot = outp.tile([P, WMAX], bf16, tag="ot")

            # First tile: edge ops in two column chunks (earlier start).
            echunks = [(0, H0), (H0, W)] if t == 0 else [(0, W)]
            for e0, e1 in echunks:
                nc.vector._custom_dve(
                    iou_edge,
                    out=rx[:, e0:e1],
                    in0=x1t[:, o + e0 : o + e1],
                    in1=x2t[:, o + e0 : o + e1],
                    s0=sct[:, c : c + 1],
                    s1=sct[:, c + 1 : c + 2],
                )
            for e0, e1 in echunks:
                nc.vector._custom_dve(
                    iou_edge,
                    out=ry[:, e0:e1],
                    in0=y1t[:, o + e0 : o + e1],
                    in1=y2t[:, o + e0 : o + e1],
                    s0=sct[:, c + 2 : c + 3],
                    s1=sct[:, c + 3 : c + 4],
                )
            nc.vector.tensor_mul(inter[:, :W], rx[:, :W], ry[:, :W])

            # p = a2e - inter on TensorE (constant stationaries), in
            # 512-col psum chunks; a1 rides the Ln bias.
            NCH = -(-W // PS)
            pts = []
            for k in range(NCH):
                c0 = k * PS
                c1 = min(W, c0 + PS)
                pt = psum.tile([P, PS], f32, tag="pt", bufs=8)
                pt = pt[:, : c1 - c0]
                pts.append((pt, c0, c1))
                nc.tensor.matmul(
                    pt[:],
                    ones1t[:],
                    a2e1t[:, o + c0 : o + c1],
                    start=True,
                    stop=False,
                )
            for pt, c0, c1 in pts:
                nc.tensor.matmul(
                    pt[:], negit[:], inter[:, c0:c1], start=False, stop=True
                )
            for pt, c0, c1 in pts:
                nc.scalar.activation(
                    ua[:, c0:c1], pt[:], act.Ln, bias=sct[:, c + 4 : c + 5]
                )

            # Last tile: split the output stage so the final store overlaps.
            if t == NT - 1:
                h = min(NCH, 3) * PS
                ochunks = [(0, min(h, W)), (min(h, W), W)]
                ochunks = [(a, b) for a, b in ochunks if b > a]
            else:
                ochunks = [(0, W)]
            for a, b in ochunks:
                nc.scalar.activation(
                    rinv[:, a:b], ua[:, a:b], act.Exp, scale=-1.0
                )
                nc.vector.tensor_mul(ot[:, a:b], inter[:, a:b], rinv[:, a:b])
                nc.sync.dma_start(out[t * P : (t + 1) * P, a:b], ot[:, a:b])

    nc.compile()
    return nc


def _get_program(WT, OFFS, WCOL):
    key = (tuple(WT), tuple(OFFS), WCOL)
    if key not in _COMPILED:
        _COMPILED[key] = _build_program(list(WT), list(OFFS), WCOL)
    return _COMPILED[key]


def _plan(boxes1, boxes2):
    """Sort boxes, derive per-tile column windows and the OFFS/WT packing."""
    b1 = np.ascontiguousarray(boxes1, dtype=np.float32)
    b2 = np.ascontiguousarray(boxes2, dtype=np.float32)
    p1 = np.argsort(b1[:, 0], kind="stable")
    p2 = np.argsort(b2[:, 0], kind="stable")
    s1 = b1[p1]
    s2 = b2[p2]
    X1 = s2[:, 0]
    wmax2 = float((s2[:, 2] - s2[:, 0]).max())

    jL = np.empty((NCORES, NT), np.int64)
    jR = np.empty((NCORES, NT), np.int64)
    for c in range(NCORES):
        for t in range(NT):
            rows = s1[c * ROWS + t * P : c * ROWS + (t + 1) * P]
            lo = float(rows[:, 0].min())
            hi = float(rows[:, 2].max())
            jL[c, t] = np.searchsorted(X1, np.float32(lo - wmax2) - 1e-3)
            jR[c, t] = np.searchsorted(X1, np.float32(hi) + 1e-3)

    def wneed(offs):
        l = jL - offs[None, :]
        r = jR - offs[None, :]
        return int((r.max(axis=1) - l.min(axis=1)).max())

    ts = np.arange(NT)
    best = None
    for S in range(0, 513, 16):
        Wn = wneed(S * ts)
        if best is None or Wn < best[0]:
            best = (Wn, S * ts)
    med = np.median(jL - jL[:, :1], axis=0)
    cand = 2 * np.round((med - med.min()) / 2).astype(np.int64)
    Wn = wneed(cand)
    if Wn < best[0]:
        best = (Wn, cand)
    _, offs = best
    offs = offs - offs.min()
    bases = (jL - offs[None, :]).min(axis=1)  # per-core packed origin
    # per-tile widths (max over cores), rounded up
    wt = (jR - offs[None, :] - bases[:, None]).max(axis=0)
    WT = [min(int(-(-max(int(w), 64) // 32) * 32), M + 512) for w in wt]
    WCOL = int(max(offs[t] + WT[t] for t in range(NT)))
    return dict(
        b1=b1, b2=b2, p1=p1, p2=p2, s1=s1, s2=s2,
        WT=WT, OFFS=[int(o) for o in offs], WCOL=WCOL, bases=bases,
    )


def _make_in_maps(plan):
    s1, s2 = plan["s1"], plan["s2"]
    WCOL, bases = plan["WCOL"], plan["bases"]

    X1, Y1, X2, Y2 = s2[:, 0], s2[:, 1], s2[:, 2], s2[:, 3]
    a2e = ((X2 - X1) * (Y2 - Y1) + np.float32(EPS)).astype(np.float32)

    in_maps = []
    for c in range(NCORES):
        idx = bases[c] + np.arange(WCOL)
        valid = (idx >= 0) & (idx < M)
        idxc = np.clip(idx, 0, M - 1)
        pad = np.float32(-1e6)

        def rep(vec, fill, dt=np.float32):
            row = np.where(valid, vec[idxc], fill).astype(dt)
            return np.ascontiguousarray(np.broadcast_to(row, (P, WCOL)))

        m = {
            "x1b": rep(X1, pad),
            "x2b": rep(X2, pad),
            "y1b": rep(Y1, pad),
            "y2b": rep(Y2, pad),
        }
        rows = s1[c * ROWS : (c + 1) * ROWS].reshape(NT, P, 4)
        a1 = (rows[:, :, 2] - rows[:, :, 0]) * (rows[:, :, 3] - rows[:, :, 1])
        scv = np.empty((P, NT * 5), dtype=np.float32)
        for t in range(NT):
            scv[:, t * 5 + 0] = rows[t, :, 0]
            scv[:, t * 5 + 1] = rows[t, :, 2]
            scv[:, t * 5 + 2] = rows[t, :, 1]
            scv[:, t * 5 + 3] = rows[t, :, 3]
            scv[:, t * 5 + 4] = a1[t]
        m["sc"] = scv
        m["a2e1"] = np.ascontiguousarray(
            np.where(valid, a2e[idxc], np.float32(1.0)).astype(np.float16)
        ).reshape(1, WCOL)
        m["ones1"] = np.ones((1, P), np.float16)
        m["negi"] = (-np.eye(P)).astype(np.float16)
        in_maps.append(m)
    return in_maps


def _assemble(plan, results):
    """Paste per-core [ROWS, WMAX] bf16 blocks into the full fp32 matrix."""
    WT, OFFS, bases = plan["WT"], plan["OFFS"], plan["bases"]
    p1, p2 = plan["p1"], plan["p2"]

    out_sorted = np.zeros((N, M), dtype=np.float32)
    for c in range(NCORES):
        blk = np.asarray(results[c]["out"])
        for t in range(NT):
            c0 = bases[c] + OFFS[t]
            c1 = c0 + WT[t]
            s0 = max(0, -c0)
            cc0 = max(0, c0)
            cc1 = min(M, c1)
            if cc1 <= cc0:
                continue
            out_sorted[
                c * ROWS + t * P : c * ROWS + (t + 1) * P, cc0:cc1
            ] = blk[t * P : (t + 1) * P, s0 : s0 + (cc1 - cc0)].astype(
                np.float32
            )

    inv1 = np.empty(N, np.int64)
    inv1[p1] = np.arange(N)
    inv2 = np.empty(M, np.int64)
    inv2[p2] = np.arange(M)
    tmp = out_sorted[inv1]
    return np.take(tmp, inv2, axis=1)


def _run(inputs, trace=False, tmpdir=None):
    from concourse.bass_utils import run_bass_kernel_spmd

    plan = _plan(inputs["boxes1"], inputs["boxes2"])
    nc = _get_program(plan["WT"], plan["OFFS"], plan["WCOL"])
    in_maps = _make_in_maps(plan)
    kwargs = {}
    if trace:
        kwargs = dict(trace=True, tmpdir=tmpdir)
    res = run_bass_kernel_spmd(
        nc, in_maps, core_ids=list(range(NCORES)), **kwargs
    )
    return plan, res


def kernel(boxes1: np.ndarray, boxes2: np.ndarray) -> np.ndarray:
    plan, res = _run({"boxes1": boxes1, "boxes2": boxes2})
    return _assemble(plan, res.results)
